# revision 1
# baseline (speedup 1.0000x reference)
"""Trainium2 Bass kernel for a deformable-DETR style decoder layer.

Strategy (8 NeuronCores): core c handles (batch b = c//2, head-group g = c%2,
heads [3g, 3g+3)).  The multi-scale deformable attention never materializes the
value projection of the whole src [S=66836, 384]; instead the sampling
locations are computed on-device, the ~4 corner rows per sample point are
fetched with indirect-gather DMA (row pairs of 2*384 floats), combined with
the (attention x bilinear) coefficients via PE matmuls against a fixed 0/1
query mask, and only then projected by the per-head slice of ms_val_w.
Head-group partials are exchanged with a 2-rank AllGather, after which each
core of a pair runs the identical tail (out-proj, LN, FFN, LN).

All per-core specialization (batch slice, head-sliced weights, sampling
tables) flows through the per-core input maps, so a single SPMD program runs
on all 8 cores.
"""
import numpy as np

B, Q, C, H, L, PTS, DFF, EXTRA = 4, 64, 384, 6, 5, 4, 1024, 128
SIZES = [(14, 14), (28, 28), (56, 56), (112, 112), (224, 224)]
S = sum(h * w for h, w in SIZES)
DH = C // H
N_CORES = 8
HG = 3                      # heads per core
NCOL = HG * L * PTS         # 60 sample columns per core
MAGIC = float(3 * 2 ** 22)  # 1.5*2^23 -> rne(x) for |x| < 2^22
GCHUNK = 4                  # gather-index columns per indirect DMA
JH = 5 * 4                  # itile columns per head (L*PTS)
WROW_B = 16660              # window B base row (start of level 4)
WROW_C = 49364              # window C base row
NW_B = WROW_C - WROW_B + 1  # 32705 rows in window B view
NW_C = 66836 - WROW_C - 1   # rows in window C view (max extent fits src)
EPS = 1e-5

_CACHE = {}


def _emit(tc, io, use_ag=True):
    """Emit the SPMD program for one core. io: dict name -> AP of dram tensors."""
    import concourse.bass as bass
    import concourse.mybir as mybir
    from concourse.masks import make_identity
    nc = tc.nc
    f32, bf16, i32 = mybir.dt.float32, mybir.dt.bfloat16, mybir.dt.int32
    AL = mybir.AluOpType
    ACT = mybir.ActivationFunctionType
    AX = mybir.AxisListType

    from contextlib import ExitStack
    stack = ExitStack()
    wpool = stack.enter_context(tc.tile_pool(name="weights", bufs=1))
    sb = stack.enter_context(tc.tile_pool(name="work", bufs=1))
    ps = stack.enter_context(tc.tile_pool(name="psum", bufs=2, space="PSUM"))
    pse = ps
    dram = stack.enter_context(tc.tile_pool(name="dram", bufs=1, space="DRAM"))

    def load_w(name, rows=128, eng=None, dtype=None):
        """DRAM weight [K, N] -> list of [<=rows, N] sbuf tiles."""
        dtype = dtype or f32
        ap = io[name]
        k, n = ap.shape
        tiles = []
        for i in range(0, k, rows):
            r = min(rows, k - i)
            t = wpool.tile([r, n], dtype, name=f"{name}_{i}", uniquify=True)
            (eng or nc.sync).dma_start(out=t[:], in_=ap[i:i + r, :])
            tiles.append(t)
        return tiles

    def load_small(names, eng):
        for name in names:
            ap = io[name]
            t = wpool.tile(list(ap.shape), f32, name=f"{name}_sb", uniquify=True)
            eng.dma_start(out=t[:], in_=ap[:])
            SMALL[name] = t

    W = {}
    SMALL = {}
    # activations + first-needed weights on the sync HWDGE ring, in use order
    tgt0 = wpool.tile([Q, C], f32, name="tgt0")
    nc.sync.dma_start(out=tgt0[:], in_=io["tgt_in"][:])
    extra = wpool.tile([EXTRA, C], f32, name="extra")
    nc.sync.dma_start(out=extra[:], in_=io["extra_in"][:])
    W["sa_in_wT"] = load_w("sa_in_wT", dtype=bf16)
    load_small(["sa_in_b", "sa_out_b", "ln2_g", "ln2_b", "refpts"], nc.sync)
    W["sa_out_wT"] = load_w("sa_out_wT", dtype=bf16)
    W["ea_in_wT"] = load_w("ea_in_wT", dtype=bf16)
    load_small(["ea_in_b", "ea_out_b", "lne_g", "lne_b"], nc.sync)
    W["ea_out_wT"] = load_w("ea_out_wT", dtype=bf16)
    W["offaw_wT"] = load_w("offaw_wT", dtype=bf16)
    load_small(["offaw_b", "xscale", "yscale", "wtab", "wm1", "wm2", "hm1",
                "hm2", "basetab"], nc.sync)
    # later-phase weights on the second HWDGE ring (scalar)
    W["val_wT_g"] = load_w("val_wT_g", eng=nc.scalar, dtype=bf16)
    load_small(["val_b_g", "out_b", "ffn_b1", "ffn_b2",
                "ln1_g", "ln1_b", "ln3_g", "ln3_b"], nc.scalar)
    W["out_wT"] = load_w("out_wT", eng=nc.scalar, dtype=bf16)
    W["ffn_w1T"] = load_w("ffn_w1T", eng=nc.scalar, dtype=bf16)
    W["ffn_w2T"] = load_w("ffn_w2T", eng=nc.scalar, dtype=bf16)

    mask_f = wpool.tile([128, Q], f32, name="mask_f")
    nc.sync.dma_start(out=mask_f[:], in_=io["mask128"][:])
    mask_bf = wpool.tile([128, Q], bf16, name="mask_bf")
    nc.vector.tensor_copy(out=mask_bf[:], in_=mask_f[:])

    ident = wpool.tile([128, 128], f32, name="ident")
    make_identity(nc, ident[:])
    ident_bf = wpool.tile([128, 128], bf16, name="ident_bf")
    nc.vector.tensor_copy(out=ident_bf[:], in_=ident[:])
    ones = wpool.tile([1, 128], f32, name="ones_row")
    nc.vector.memset(ones[:], 1.0)
    zcol = wpool.tile([128, 1], f32, name="zcol")
    nc.vector.memset(zcol[:], 0.0)
    epscol = wpool.tile([128, 1], f32, name="epscol")
    nc.vector.memset(epscol[:], EPS)
    ones_bf = wpool.tile([1, 128], bf16, name="ones_bf")
    nc.vector.memset(ones_bf[:], 1.0)
    BF_B = {}
    for bname in ["val_b_g", "out_b", "ffn_b1", "ffn_b2", "sa_in_b",
                  "sa_out_b", "ea_in_b", "ea_out_b", "offaw_b"]:
        t = wpool.tile(list(io[bname].shape), bf16, name=f"{bname}_bf", uniquify=True)
        nc.vector.tensor_copy(out=t[:], in_=SMALL[bname][:])
        BF_B[bname] = t

    def transpose_tiles(x, tag, dtype=None):
        """x: sbuf AP [p<=128, f] -> list of sbuf tiles [ck, p] (f in chunks of 128)."""
        dtype = dtype or f32
        idn = ident if dtype == f32 else ident_bf
        p, f = x.shape[0], x.shape[1]
        outs = []
        for i in range(0, f, 128):
            ck = min(128, f - i)
            pt = pse.tile([128, 128], dtype, name=f"{tag}_tp_{i}", uniquify=True, tag="tp")
            nc.tensor.transpose(out=pt[:ck, :p], in_=x[:, i:i + ck], identity=idn[:p, :p])
            st = sb.tile([ck, p], dtype, name=f"{tag}_t_{i}", uniquify=True, tag=f"{tag}_t{i}")
            nc.vector.tensor_copy(out=st[:], in_=pt[:ck, :p])
            outs.append(st)
        return outs

    def mm_acc(out_ps, lhsT_tiles, rhs_tiles_cols, bias=None, nq=Q):
        """out_ps += sum_i lhsT_tiles[i].T @ rhs_cols[i]; optional (+ ones^T bias)."""
        n = len(lhsT_tiles)
        for i in range(n):
            nc.tensor.matmul(out_ps, lhsT=lhsT_tiles[i][:], rhs=rhs_tiles_cols[i],
                             start=(i == 0), stop=(bias is None and i == n - 1))
        if bias is not None:
            nc.tensor.matmul(out_ps, lhsT=ones[:1, :nq], rhs=bias,
                             start=False, stop=True)

    def ln(x_res_a, x_res_b, gname, bname, tag):
        """LayerNorm(a + b) -> new sbuf tile [Q, C]."""
        xs = sb.tile([Q, C], f32, name=f"{tag}_x", uniquify=True, tag=f"{tag}_x")
        nc.vector.tensor_tensor(out=xs[:], in0=x_res_a, in1=x_res_b, op=AL.add)
        mu = sb.tile([Q, 1], f32, name=f"{tag}_mu", uniquify=True, tag=f"{tag}_mu")
        nc.vector.reduce_sum(out=mu[:], in_=xs[:], axis=AX.X)
        nc.vector.tensor_scalar(out=mu[:], in0=mu[:], scalar1=-1.0 / C, scalar2=None,
                                op0=AL.mult)
        nc.vector.tensor_scalar(out=xs[:], in0=xs[:], scalar1=mu[:, :1], scalar2=None,
                                op0=AL.add)
        sq = sb.tile([Q, C], f32, name=f"{tag}_sq", uniquify=True, tag=f"{tag}_sq")
        nc.vector.tensor_tensor(out=sq[:], in0=xs[:], in1=xs[:], op=AL.mult)
        var = sb.tile([Q, 1], f32, name=f"{tag}_var", uniquify=True, tag=f"{tag}_var")
        nc.vector.reduce_sum(out=var[:], in_=sq[:], axis=AX.X)
        std = sb.tile([Q, 1], f32, name=f"{tag}_std", uniquify=True, tag=f"{tag}_std")
        nc.scalar.activation(out=std[:], in_=var[:], func=ACT.Sqrt,
                             bias=epscol[:Q, :1], scale=1.0 / C)
        rstd = sb.tile([Q, 1], f32, name=f"{tag}_rstd", uniquify=True, tag=f"{tag}_rstd")
        nc.vector.reciprocal(out=rstd[:], in_=std[:])
        nc.vector.tensor_scalar(out=xs[:], in0=xs[:], scalar1=rstd[:, :1], scalar2=None,
                                op0=AL.mult)
        nc.vector.tensor_tensor(out=xs[:], in0=xs[:], in1=SMALL[gname][:], op=AL.mult)
        nc.vector.tensor_tensor(out=xs[:], in0=xs[:], in1=SMALL[bname][:], op=AL.add)
        return xs

    def softmax_rows(src_ps, out_sb, ncols, scale=1.0, tag="sm"):
        """out_sb[:, :ncols] = softmax(src_ps * scale) along free axis."""
        nq = out_sb.shape[0]
        m = sb.tile([nq, 1], f32, name=f"{tag}_m", uniquify=True, tag=f"{tag}_m")
        nc.vector.reduce_max(out=m[:], in_=src_ps, axis=AX.X)
        nc.vector.tensor_scalar(out=m[:], in0=m[:], scalar1=-scale, scalar2=None,
                                op0=AL.mult)
        nc.scalar.activation(out=out_sb, in_=src_ps, func=ACT.Exp,
                             bias=m[:, :1], scale=scale)
        ssum = sb.tile([nq, 1], f32, name=f"{tag}_s", uniquify=True, tag=f"{tag}_s")
        nc.vector.reduce_sum(out=ssum[:], in_=out_sb, axis=AX.X)
        rinv = sb.tile([nq, 1], f32, name=f"{tag}_r", uniquify=True, tag=f"{tag}_r")
        nc.vector.reciprocal(out=rinv[:], in_=ssum[:])
        nc.vector.tensor_scalar(out=out_sb, in0=out_sb, scalar1=rinv[:, :1],
                                scalar2=None, op0=AL.mult)

    # ---------------- phase 1: self attention ----------------
    def attention(x_bf, kv_bf, nk, wT, b_in_bf, woT, b_out_bf, tag):
        """MHA in bf16 (f32 psum). Returns out-proj psum [Q, C] f32."""
        xT = transpose_tiles(x_bf[:], f"{tag}_x", dtype=bf16)    # 3 x [128, Q]
        kvT = xT if kv_bf is x_bf else transpose_tiles(kv_bf[:], f"{tag}_kv",
                                                       dtype=bf16)
        # V [nk, C]
        v_ps = ps.tile([nk, C], f32, name=f"{tag}_vps", uniquify=True, tag="acc")
        for i in range(3):
            nc.tensor.matmul(v_ps[:], lhsT=kvT[i][:], rhs=wT[i][:, 2 * C:3 * C],
                             start=(i == 0), stop=False)
        nc.tensor.matmul(v_ps[:], lhsT=ones_bf[:1, :nk],
                         rhs=b_in_bf[:1, 2 * C:3 * C], start=False, stop=True)
        v_sb = sb.tile([nk, C], bf16, name=f"{tag}_v", uniquify=True, tag=f"{tag}_v")
        nc.vector.tensor_copy(out=v_sb[:], in_=v_ps[:])
        # qT chunks [128, Q]; kT chunks [128, nk]
        qT = []
        for m in range(3):
            t_ps = pse.tile([128, Q], f32, name=f"{tag}_qTp{m}", uniquify=True, tag="mm")
            for i in range(3):
                nc.tensor.matmul(t_ps[:], lhsT=wT[i][:, m * 128:(m + 1) * 128],
                                 rhs=xT[i][:], start=(i == 0), stop=False)
            nc.tensor.matmul(t_ps[:], lhsT=b_in_bf[:1, m * 128:(m + 1) * 128],
                             rhs=ones_bf[:1, :Q], start=False, stop=True)
            t_sb = sb.tile([128, Q], bf16, name=f"{tag}_qT{m}", uniquify=True, tag=f"{tag}_qT{m}")
            nc.vector.tensor_copy(out=t_sb[:], in_=t_ps[:])
            qT.append(t_sb)
        kT = []
        for m in range(3):
            t_ps = pse.tile([128, nk], f32, name=f"{tag}_kTp{m}", uniquify=True, tag="mm")
            for i in range(3):
                nc.tensor.matmul(t_ps[:], lhsT=wT[i][:, C + m * 128:C + (m + 1) * 128],
                                 rhs=kvT[i][:], start=(i == 0), stop=False)
            nc.tensor.matmul(t_ps[:], lhsT=b_in_bf[:1, C + m * 128:C + (m + 1) * 128],
                             rhs=ones_bf[:1, :nk], start=False, stop=True)
            t_sb = sb.tile([128, nk], bf16, name=f"{tag}_kT{m}", uniquify=True, tag=f"{tag}_kT{m}")
            nc.vector.tensor_copy(out=t_sb[:], in_=t_ps[:])
            kT.append(t_sb)
        # per-head scores + softmax + o
        o_ps = ps.tile([Q, C], f32, name=f"{tag}_ops", uniquify=True, tag="acc")
        for h in range(H):
            m, p0 = h // 2, (h % 2) * 64
            s_ps = pse.tile([Q, nk], f32, name=f"{tag}_s{h}", uniquify=True, tag="mm")
            nc.tensor.matmul(s_ps[:], lhsT=qT[m][p0:p0 + 64, :],
                             rhs=kT[m][p0:p0 + 64, :], start=True, stop=True)
            mx = sb.tile([Q, 1], f32, name=f"{tag}_m{h}", uniquify=True, tag=f"{tag}_m")
            nc.vector.reduce_max(out=mx[:], in_=s_ps[:], axis=AX.X)
            nc.vector.tensor_scalar(out=mx[:], in0=mx[:], scalar1=-1.0 / np.sqrt(DH),
                                    scalar2=None, op0=AL.mult)
            e_sb = sb.tile([Q, nk], f32, name=f"{tag}_e{h}", uniquify=True,
                           tag=f"{tag}_e", bufs=2)
            nc.scalar.activation(out=e_sb[:], in_=s_ps[:], func=ACT.Exp,
                                 bias=mx[:, :1], scale=1.0 / np.sqrt(DH))
            ssum = sb.tile([Q, 1], f32, name=f"{tag}_ss{h}", uniquify=True, tag=f"{tag}_ss")
            nc.vector.reduce_sum(out=ssum[:], in_=e_sb[:], axis=AX.X)
            rinv = sb.tile([Q, 1], f32, name=f"{tag}_ri{h}", uniquify=True, tag=f"{tag}_ri")
            nc.vector.reciprocal(out=rinv[:], in_=ssum[:])
            a_sb = sb.tile([Q, nk], bf16, name=f"{tag}_a{h}", uniquify=True,
                           tag=f"{tag}_a", bufs=2)
            nc.vector.tensor_scalar(out=a_sb[:], in0=e_sb[:], scalar1=rinv[:, :1],
                                    scalar2=None, op0=AL.mult)
            aT_ps = pse.tile([nk, Q], bf16, name=f"{tag}_aTp{h}", uniquify=True, tag="tp")
            nc.tensor.transpose(out=aT_ps[:], in_=a_sb[:], identity=ident_bf[:Q, :Q])
            aT_sb = sb.tile([nk, Q], bf16, name=f"{tag}_aT{h}", uniquify=True,
                            tag=f"{tag}_aT", bufs=2)
            nc.vector.tensor_copy(out=aT_sb[:], in_=aT_ps[:])
            nc.tensor.matmul(o_ps[:, h * DH:(h + 1) * DH], lhsT=aT_sb[:],
                             rhs=v_sb[:, h * DH:(h + 1) * DH], start=True, stop=True)
        o_sb = sb.tile([Q, C], bf16, name=f"{tag}_o", uniquify=True, tag=f"{tag}_o")
        nc.vector.tensor_copy(out=o_sb[:], in_=o_ps[:])
        oT = transpose_tiles(o_sb[:], f"{tag}_o", dtype=bf16)
        t2_ps = ps.tile([Q, C], f32, name=f"{tag}_t2", uniquify=True, tag="acc")
        for i in range(3):
            nc.tensor.matmul(t2_ps[:], lhsT=oT[i][:], rhs=woT[i][:, :],
                             start=(i == 0), stop=False)
        nc.tensor.matmul(t2_ps[:], lhsT=ones_bf[:1, :Q], rhs=b_out_bf[:1, :],
                         start=False, stop=True)
        return t2_ps

    tgt0_bf = sb.tile([Q, C], bf16, name="tgt0_bf")
    nc.vector.tensor_copy(out=tgt0_bf[:], in_=tgt0[:])
    extra_bf = sb.tile([EXTRA, C], bf16, name="extra_bf")
    nc.vector.tensor_copy(out=extra_bf[:], in_=extra[:])
    t2 = attention(tgt0_bf, tgt0_bf, Q, W["sa_in_wT"], BF_B["sa_in_b"][:],
                   W["sa_out_wT"], BF_B["sa_out_b"][:], "sa")
    tgt1 = ln(tgt0[:], t2[:], "ln2_g", "ln2_b", "ln2")
    tgt1_bf = sb.tile([Q, C], bf16, name="tgt1_bf")
    nc.vector.tensor_copy(out=tgt1_bf[:], in_=tgt1[:])
    t2 = attention(tgt1_bf, extra_bf, EXTRA, W["ea_in_wT"], BF_B["ea_in_b"][:],
                   W["ea_out_wT"], BF_B["ea_out_b"][:], "ea")
    tgt2 = ln(tgt1[:], t2[:], "lne_g", "lne_b", "lne")

    # ---------------- phase 2: sampling locations ----------------
    tgt2_bf = sb.tile([Q, C], bf16, name="tgt2_bf")
    nc.vector.tensor_copy(out=tgt2_bf[:], in_=tgt2[:])
    tgt2T = transpose_tiles(tgt2_bf[:], "t2T", dtype=bf16)
    oa_ps = ps.tile([Q, 3 * NCOL], f32, name="oa_ps", tag="acc")
    for i in range(3):
        nc.tensor.matmul(oa_ps[:], lhsT=tgt2T[i][:], rhs=W["offaw_wT"][i][:, :],
                         start=(i == 0), stop=False)
    nc.tensor.matmul(oa_ps[:], lhsT=ones_bf[:1, :Q], rhs=BF_B["offaw_b"][:1, :],
                     start=False, stop=True)
    oa = sb.tile([Q, 3 * NCOL], f32, name="oa")
    nc.vector.tensor_copy(out=oa[:], in_=oa_ps[:])
    aw = sb.tile([Q, NCOL], f32, name="aw")
    for h in range(HG):
        c0 = 2 * NCOL + h * L * PTS
        softmax_rows(oa[:, c0:c0 + L * PTS], aw[:, h * L * PTS:(h + 1) * L * PTS],
                     L * PTS, tag=f"awsm{h}")

    def floor_pair(val, tag):
        rne = sb.tile([Q, NCOL], f32, name=f"{tag}_rne", uniquify=True, tag=f"{tag}_rne")
        nc.vector.tensor_scalar(out=rne[:], in0=val, scalar1=MAGIC, scalar2=None,
                                op0=AL.add)
        nc.vector.tensor_scalar(out=rne[:], in0=rne[:], scalar1=-MAGIC, scalar2=None,
                                op0=AL.add)
        gt = sb.tile([Q, NCOL], f32, name=f"{tag}_gt", uniquify=True, tag=f"{tag}_gt")
        nc.vector.tensor_tensor(out=gt[:], in0=rne[:], in1=val, op=AL.is_gt)
        fl = sb.tile([Q, NCOL], f32, name=f"{tag}_fl", uniquify=True, tag=f"{tag}_fl")
        nc.vector.tensor_tensor(out=fl[:], in0=rne[:], in1=gt[:], op=AL.subtract)
        fr = sb.tile([Q, NCOL], f32, name=f"{tag}_fr", uniquify=True, tag=f"{tag}_fr")
        nc.vector.tensor_tensor(out=fr[:], in0=val, in1=fl[:], op=AL.subtract)
        return fl, fr

    def slot_weights(coord_tab, m1tab, m2tab, refcol, offv, tag):
        """x = ref*scale + off - 0.5; returns (clipped start, w_slot0, w_slot1)."""
        x = sb.tile([Q, NCOL], f32, name=f"{tag}_x", uniquify=True, tag=f"{tag}_x")
        nc.vector.tensor_scalar(out=x[:], in0=coord_tab, scalar1=refcol,
                                scalar2=-0.5, op0=AL.mult, op1=AL.add)
        nc.vector.tensor_tensor(out=x[:], in0=x[:], in1=offv, op=AL.add)
        x0, wx = floor_pair(x[:], f"{tag}_f")
        xs = sb.tile([Q, NCOL], f32, name=f"{tag}_xs", uniquify=True, tag=f"{tag}_xs")
        nc.vector.tensor_scalar(out=xs[:], in0=x0[:], scalar1=0.0, scalar2=None,
                                op0=AL.max)
        nc.vector.tensor_tensor(out=xs[:], in0=xs[:], in1=m2tab, op=AL.min)
        ge = sb.tile([Q, NCOL], f32, name=f"{tag}_ge", uniquify=True, tag=f"{tag}_ge")
        nc.vector.tensor_scalar(out=ge[:], in0=x0[:], scalar1=0.0, scalar2=None,
                                op0=AL.is_ge)
        le = sb.tile([Q, NCOL], f32, name=f"{tag}_le", uniquify=True, tag=f"{tag}_le")
        nc.vector.tensor_tensor(out=le[:], in0=x0[:], in1=m2tab, op=AL.is_le)
        inb = sb.tile([Q, NCOL], f32, name=f"{tag}_in", uniquify=True, tag=f"{tag}_in")
        nc.vector.tensor_tensor(out=inb[:], in0=ge[:], in1=le[:], op=AL.mult)
        em1 = sb.tile([Q, NCOL], f32, name=f"{tag}_em1", uniquify=True, tag=f"{tag}_em1")
        nc.vector.tensor_scalar(out=em1[:], in0=x0[:], scalar1=-1.0, scalar2=None,
                                op0=AL.is_equal)
        eW = sb.tile([Q, NCOL], f32, name=f"{tag}_eW", uniquify=True, tag=f"{tag}_eW")
        nc.vector.tensor_tensor(out=eW[:], in0=x0[:], in1=m1tab, op=AL.is_equal)
        onemw = sb.tile([Q, NCOL], f32, name=f"{tag}_omw", uniquify=True, tag=f"{tag}_omw")
        nc.vector.tensor_scalar(out=onemw[:], in0=wx[:], scalar1=-1.0, scalar2=1.0,
                                op0=AL.mult, op1=AL.add)
        w0 = sb.tile([Q, NCOL], f32, name=f"{tag}_w0", uniquify=True, tag=f"{tag}_w0")
        nc.vector.tensor_tensor(out=w0[:], in0=onemw[:], in1=inb[:], op=AL.mult)
        tmp = sb.tile([Q, NCOL], f32, name=f"{tag}_tmp", uniquify=True, tag=f"{tag}_tmp")
        nc.vector.tensor_tensor(out=tmp[:], in0=wx[:], in1=em1[:], op=AL.mult)
        nc.vector.tensor_tensor(out=w0[:], in0=w0[:], in1=tmp[:], op=AL.add)
        w1 = sb.tile([Q, NCOL], f32, name=f"{tag}_w1", uniquify=True, tag=f"{tag}_w1")
        nc.vector.tensor_tensor(out=w1[:], in0=wx[:], in1=inb[:], op=AL.mult)
        nc.vector.tensor_tensor(out=tmp[:], in0=onemw[:], in1=eW[:], op=AL.mult)
        nc.vector.tensor_tensor(out=w1[:], in0=w1[:], in1=tmp[:], op=AL.add)
        return xs, w0, w1

    off4 = oa[:, :2 * NCOL].rearrange("q (c two) -> q c two", two=2)
    xs, wxs0, wxs1 = slot_weights(SMALL["xscale"][:], SMALL["wm1"][:], SMALL["wm2"][:],
                                  SMALL["refpts"][:, 0:1], off4[:, :, 0], "sx")
    ys, wys0, wys1 = slot_weights(SMALL["yscale"][:], SMALL["hm1"][:], SMALL["hm2"][:],
                                  SMALL["refpts"][:, 1:2], off4[:, :, 1], "sy")

    # row indices: imat [Q, 2*NCOL]; col c*2+v
    imat = sb.tile([Q, 2 * NCOL], f32, name="imat")
    i3 = imat[:].rearrange("q (c v) -> q c v", v=2)
    nc.vector.tensor_tensor(out=i3[:, :, 0], in0=ys[:], in1=SMALL["wtab"][:], op=AL.mult)
    nc.vector.tensor_tensor(out=i3[:, :, 0], in0=i3[:, :, 0], in1=xs[:], op=AL.add)
    nc.vector.tensor_tensor(out=i3[:, :, 0], in0=i3[:, :, 0], in1=SMALL["basetab"][:],
                            op=AL.add)
    nc.vector.tensor_tensor(out=i3[:, :, 1], in0=i3[:, :, 0], in1=SMALL["wtab"][:],
                            op=AL.add)
    # coefficients: cmat [Q, 4*NCOL]; col c*4 + v*2 + u
    cmat = sb.tile([Q, 4 * NCOL], f32, name="cmat")
    c4 = cmat[:].rearrange("q (c v u) -> q c v u", v=2, u=2)
    wvu = sb.tile([Q, NCOL], f32, name="wvu")
    for v, wv in enumerate([wys0, wys1]):
        for u, wu in enumerate([wxs0, wxs1]):
            nc.vector.tensor_tensor(out=wvu[:], in0=wv[:], in1=wu[:], op=AL.mult)
            nc.vector.tensor_tensor(out=c4[:, :, v, u], in0=wvu[:], in1=aw[:],
                                    op=AL.mult)

    # ---- shuffle to level-pure columns: (p = s*64+q, col j) <-> jk = 2j+s ----
    # ctile[s*64+q, h*40 + 2j+u] = cmat[q, h*80 + 4j + 2s + u]
    ctile = sb.tile([128, 2 * NCOL], f32, name="ctile")
    cm5 = cmat[:].rearrange("q (h j v u) -> q h j v u", h=HG, j=JH, v=2)
    for hh in range(HG):
        nc.vector.tensor_copy(
            out=ctile[0:64, hh * 2 * JH:(hh + 1) * 2 * JH].rearrange(
                "q (j u) -> q j u", j=JH),
            in_=cm5[:, hh, :, 0, :])
        nc.sync.dma_start(
            out=ctile[64:128, hh * 2 * JH:(hh + 1) * 2 * JH].rearrange(
                "q (j u) -> q j u", j=JH),
            in_=cm5[:, hh, :, 1, :])
    # ifl[s*64+q, h*20+j] = imat[q, h*40 + 2j + s]
    ifl = sb.tile([128, NCOL], f32, name="ifl")
    im4 = imat[:].rearrange("q (h j v) -> q h j v", h=HG, j=JH)
    for hh in range(HG):
        nc.vector.tensor_copy(out=ifl[0:64, hh * JH:(hh + 1) * JH],
                              in_=im4[:, hh, :, 0])
        nc.scalar.dma_start(out=ifl[64:128, hh * JH:(hh + 1) * JH],
                            in_=im4[:, hh, :, 1])
    # ---- level-4 window splitting (int16 windows for dma_gather) ----
    # windows: A rows [0, 16660) (levels 0-3); B rows [16660, 49364);
    #          C rows [49364, 66836). l4 points contribute to both B and C
    #          with complementary gates; clipped rel indices keep reads valid.
    i4 = im4[:, :, 16:20, :]                       # [64, HG, 4, 2] l4 row starts
    gB = sb.tile([Q, HG * 8], f32, name="gB")      # (h, j4, v): row < 49364
    g4 = gB[:].rearrange("q (h j v) -> q h j v", h=HG, j=4)
    nc.vector.tensor_scalar(out=g4, in0=i4, scalar1=float(WROW_C), scalar2=None,
                            op0=AL.is_lt)
    imatB = sb.tile([Q, HG * 8], f32, name="imatB")
    b4 = imatB[:].rearrange("q (h j v) -> q h j v", h=HG, j=4)
    nc.vector.tensor_scalar(out=b4, in0=i4, scalar1=-float(WROW_B),
                            scalar2=0.0, op0=AL.add, op1=AL.max)
    nc.vector.tensor_scalar(out=b4, in0=b4, scalar1=float(NW_B - 1), scalar2=None,
                            op0=AL.min)
    imatC = sb.tile([Q, HG * 8], f32, name="imatC")
    c4i = imatC[:].rearrange("q (h j v) -> q h j v", h=HG, j=4)
    nc.vector.tensor_scalar(out=c4i, in0=i4, scalar1=-float(WROW_C),
                            scalar2=0.0, op0=AL.add, op1=AL.max)
    nc.vector.tensor_scalar(out=c4i, in0=c4i, scalar1=float(NW_C - 1), scalar2=None,
                            op0=AL.min)
    # gated l4 coefficients: cmatB = c*gate, cmatC = c - cmatB
    cmatB = sb.tile([Q, HG * 16], f32, name="cmatB")
    cmatC = sb.tile([Q, HG * 16], f32, name="cmatC")
    cB5 = cmatB[:].rearrange("q (h j v u) -> q h j v u", h=HG, j=4, v=2)
    cC5 = cmatC[:].rearrange("q (h j v u) -> q h j v u", h=HG, j=4, v=2)
    cm_l4 = cm5[:, :, 16:20, :, :]
    for u in range(2):
        nc.vector.tensor_tensor(out=cB5[:, :, :, :, u], in0=cm_l4[:, :, :, :, u],
                                in1=g4, op=AL.mult)
        nc.vector.tensor_tensor(out=cC5[:, :, :, :, u], in0=cm_l4[:, :, :, :, u],
                                in1=cB5[:, :, :, :, u], op=AL.subtract)
    # shuffle gated l4 coeffs/indices into (p = s*64+q) layout
    ctB = sb.tile([128, HG * 8], f32, name="ctB")   # col (h, j4, u)
    ctC = sb.tile([128, HG * 8], f32, name="ctC")
    iflB = sb.tile([128, HG * 4], f32, name="iflB")  # col (h, j4)
    iflC = sb.tile([128, HG * 4], f32, name="iflC")
    for hh in range(HG):
        for (ctx, cmx) in ((ctB, cB5), (ctC, cC5)):
            nc.vector.tensor_copy(
                out=ctx[0:64, hh * 8:(hh + 1) * 8].rearrange("q (j u) -> q j u", j=4),
                in_=cmx[:, hh, :, 0, :])
            nc.sync.dma_start(
                out=ctx[64:128, hh * 8:(hh + 1) * 8].rearrange("q (j u) -> q j u", j=4),
                in_=cmx[:, hh, :, 1, :])
        for (ifx, imx) in ((iflB, b4), (iflC, c4i)):
            nc.vector.tensor_copy(out=ifx[0:64, hh * 4:(hh + 1) * 4],
                                  in_=imx[:, hh, :, 0])
            nc.scalar.dma_start(out=ifx[64:128, hh * 4:(hh + 1) * 4],
                                in_=imx[:, hh, :, 1])
    # ---- wrap indices into dma_gather layout [128, ncols*8] int16 ----
    # position i = b*128 + p  ->  idx16[rep*16 + i%16, (i//128)*8 + (i%128)//16]
    replpat = wpool.tile([16, 128], f32, name="replpat")
    nc.sync.dma_start(out=replpat[:], in_=io["replpat"][:])

    def wrap_idx(src128, ncols, tag):
        wrapf = sb.tile([16, ncols * 8], f32, name=f"wrap_{tag}", uniquify=True,
                        tag=f"wrap_{tag}")
        w3 = wrapf[:].rearrange("r (j pc) -> r j pc", j=ncols)
        for pc in range(8):
            (nc.sync if pc % 2 == 0 else nc.scalar).dma_start(
                out=w3[:, :, pc], in_=src128[pc * 16:(pc + 1) * 16, :])
        rp = pse.tile([128, ncols * 8], f32, name=f"wrp_{tag}", uniquify=True,
                      tag="mm")
        nc.tensor.matmul(rp[:], lhsT=replpat[:], rhs=wrapf[:], start=True, stop=True)
        i16 = sb.tile([128, ncols * 8], mybir.dt.int16, name=f"i16_{tag}",
                      uniquify=True, tag=f"i16_{tag}")
        nc.vector.tensor_copy(out=i16[:], in_=rp[:])
        return i16

    idxA = wrap_idx(ifl[:], NCOL, "A")         # cols (h*20+j)*8.. ; l4 cols unused
    idxB = wrap_idx(iflB[:], HG * 4, "B")
    idxC = wrap_idx(iflC[:], HG * 4, "C")

    # ---------------- phase 3: gather + combine + per-head projection ----------
    heads_sb = sb.tile([Q, HG * DH], f32, name="heads_sb")
    for h in range(HG):
        oh_ps = ps.tile([Q, C], f32, name=f"oh_ps{h}", uniquify=True, tag="oh",
                        bufs=2)
        nmm = 0
        NMM = 2 * 16 + 2 * 4 + 2 * 4

        def combine(gt, nb, coef, colbase):
            nonlocal nmm
            g3 = gt[:].rearrange("p (b e) -> p b e", e=2 * C)
            for b in range(nb):
                for u in range(2):
                    ct = sb.tile([128, Q], bf16, name=f"ct{h}_{b}_{u}",
                                 uniquify=True, tag="ct", bufs=4)
                    nc.vector.tensor_scalar(
                        out=ct[:], in0=mask_bf[:],
                        scalar1=coef[:, colbase + 2 * b + u:colbase + 2 * b + u + 1],
                        scalar2=None, op0=AL.mult)
                    nc.tensor.matmul(oh_ps[:], lhsT=ct[:],
                                     rhs=g3[:, b, u * C:(u + 1) * C],
                                     start=(nmm == 0), stop=(nmm == NMM - 1))
                    nmm += 1

        for half in range(2):
            ga = sb.tile([128, 8 * 2 * C], bf16, name=f"gA{h}_{half}",
                         uniquify=True, tag="gA", bufs=2)
            nc.gpsimd.dma_gather(
                out_ap=ga[:].rearrange("p (b e) -> p b e", e=2 * C),
                in_ap=bass.AP(io["srcflat_bf"].tensor, 0, [[C, WROW_B], [1, 2 * C]]),
                idxs_ap=idxA[:, (h * JH + 8 * half) * 8:(h * JH + 8 * half + 8) * 8],
                num_idxs=1024, num_idxs_reg=1024, elem_size=2 * C, elem_step=C)
            combine(ga, 8, ctile, h * 2 * JH + 16 * half)
        for (idxt, base, nwin, coef) in ((idxB, WROW_B, NW_B, ctB),
                                         (idxC, WROW_C, NW_C, ctC)):
            gw = sb.tile([128, 4 * 2 * C], bf16, name=f"gW{h}", uniquify=True,
                         tag="gW", bufs=2)
            nc.gpsimd.dma_gather(
                out_ap=gw[:].rearrange("p (b e) -> p b e", e=2 * C),
                in_ap=bass.AP(io["srcflat_bf"].tensor, base * C,
                              [[C, nwin], [1, 2 * C]]),
                idxs_ap=idxt[:, (h * 4) * 8:(h * 4 + 4) * 8],
                num_idxs=512, num_idxs_reg=512, elem_size=2 * C, elem_step=C)
            combine(gw, 4, coef, h * 8)
        # sum of coefficients (for value-bias correction): swT [1, Q]
        red = sb.tile([128, 1], f32, name=f"red{h}", uniquify=True, tag="red")
        nc.vector.reduce_sum(out=red[:], in_=ctile[:, h * 2 * JH:(h + 1) * 2 * JH],
                             axis=AX.X)
        swT_ps = pse.tile([1, Q], f32, name=f"swTp{h}", uniquify=True, tag="mm")
        nc.tensor.matmul(swT_ps[:], lhsT=red[:], rhs=mask_f[:], start=True, stop=True)
        swT = sb.tile([1, Q], bf16, name=f"swT{h}", uniquify=True, tag="swT")
        nc.vector.tensor_copy(out=swT[:], in_=swT_ps[:])
        oh_sb = sb.tile([Q, C], bf16, name=f"oh_sb{h}", uniquify=True, tag="oh_sb")
        nc.vector.tensor_copy(out=oh_sb[:], in_=oh_ps[:])
        ohT = transpose_tiles(oh_sb[:], f"ohT{h}", dtype=bf16)
        pj_ps = pse.tile([Q, DH], f32, name=f"pj{h}", uniquify=True, tag="mm")
        for i in range(3):
            nc.tensor.matmul(pj_ps[:], lhsT=ohT[i][:],
                             rhs=W["val_wT_g"][i][:, h * DH:(h + 1) * DH],
                             start=(i == 0), stop=False)
        nc.tensor.matmul(pj_ps[:], lhsT=swT[:1, :],
                         rhs=BF_B["val_b_g"][:1, h * DH:(h + 1) * DH],
                         start=False, stop=True)
        nc.vector.tensor_copy(out=heads_sb[:, h * DH:(h + 1) * DH], in_=pj_ps[:])

    # ---------------- phase 4: exchange head groups (2-rank AllGather) ---------
    headsfull = sb.tile([Q, C], f32, name="headsfull")
    if use_ag:
        cc_in = dram.tile([Q, HG * DH], f32, name="cc_in")
        cc_out = dram.tile([2 * Q, HG * DH], f32, name="cc_out")
        nc.gpsimd.dma_start(out=cc_in[:], in_=heads_sb[:])
        nc.gpsimd.collective_compute(
            "AllGather", mybir.AluOpType.bypass,
            replica_groups=[[0, 1], [2, 3], [4, 5], [6, 7]],
            ins=[cc_in[:].opt()], outs=[cc_out[:].opt()])
        nc.sync.dma_start(out=headsfull[:, 0:HG * DH], in_=cc_out[0:Q, :])
        nc.sync.dma_start(out=headsfull[:, HG * DH:C], in_=cc_out[Q:2 * Q, :])
    else:
        nc.vector.tensor_copy(out=headsfull[:, 0:HG * DH], in_=heads_sb[:])
        nc.vector.tensor_copy(out=headsfull[:, HG * DH:C], in_=heads_sb[:])

    # ---------------- phase 5: output proj + LN + FFN + LN ----------------
    hf_bf = sb.tile([Q, C], bf16, name="hf_bf")
    nc.vector.tensor_copy(out=hf_bf[:], in_=headsfull[:])
    hfT = transpose_tiles(hf_bf[:], "hfT", dtype=bf16)
    mo_ps = ps.tile([Q, C], f32, name="mo_ps", tag="acc")
    for i in range(3):
        nc.tensor.matmul(mo_ps[:], lhsT=hfT[i][:], rhs=W["out_wT"][i][:, :],
                         start=(i == 0), stop=False)
    nc.tensor.matmul(mo_ps[:], lhsT=ones_bf[:1, :Q], rhs=BF_B["out_b"][:1, :],
                     start=False, stop=True)
    tgt3 = ln(tgt2[:], mo_ps[:], "ln1_g", "ln1_b", "ln1")

    tgt3_bf = sb.tile([Q, C], bf16, name="tgt3_bf")
    nc.vector.tensor_copy(out=tgt3_bf[:], in_=tgt3[:])
    tgt3T = transpose_tiles(tgt3_bf[:], "t3T", dtype=bf16)
    h1 = sb.tile([Q, DFF], bf16, name="h1")
    for m in range(2):
        f1_ps = pse.tile([Q, DFF // 2], f32, name=f"f1_{m}", uniquify=True, tag="mm")
        for i in range(3):
            nc.tensor.matmul(f1_ps[:], lhsT=tgt3T[i][:],
                             rhs=W["ffn_w1T"][i][:, m * 512:(m + 1) * 512],
                             start=(i == 0), stop=False)
        nc.tensor.matmul(f1_ps[:], lhsT=ones_bf[:1, :Q],
                         rhs=BF_B["ffn_b1"][:1, m * 512:(m + 1) * 512],
                         start=False, stop=True)
        nc.scalar.activation(out=h1[:, m * 512:(m + 1) * 512], in_=f1_ps[:],
                             func=ACT.Relu, bias=zcol[:Q, :1])
    h1T = transpose_tiles(h1[:], "h1T", dtype=bf16)
    f2_ps = ps.tile([Q, C], f32, name="f2_ps", tag="acc")
    for i in range(8):
        nc.tensor.matmul(f2_ps[:], lhsT=h1T[i][:], rhs=W["ffn_w2T"][i][:, :],
                         start=(i == 0), stop=False)
    nc.tensor.matmul(f2_ps[:], lhsT=ones_bf[:1, :Q], rhs=BF_B["ffn_b2"][:1, :],
                     start=False, stop=True)
    out_sb = ln(tgt3[:], f2_ps[:], "ln3_g", "ln3_b", "ln3")
    nc.sync.dma_start(out=io["out"][:], in_=out_sb[:])
    stack.close()


def _build(n_devices=N_CORES, use_ag=True, loop=1):
    import concourse.bacc as bacc
    import concourse.mybir as mybir
    import concourse.tile as tile
    from concourse._compat import axon_active
    f32 = mybir.dt.float32
    nc = bacc.Bacc("TRN2", target_bir_lowering=False, debug=not axon_active(),
                   num_devices=n_devices)
    shapes = dict(
        tgt_in=[Q, C], extra_in=[EXTRA, C], refpts=[Q, 2],
        sa_in_b=[1, 3 * C], sa_out_b=[1, C],
        ea_in_b=[1, 3 * C], ea_out_b=[1, C],
        offaw_b=[1, 3 * NCOL],
        val_b_g=[1, HG * DH], out_b=[1, C],
        ffn_b1=[1, DFF], ffn_b2=[1, C],
        ln2_g=[Q, C], ln2_b=[Q, C], lne_g=[Q, C], lne_b=[Q, C],
        ln1_g=[Q, C], ln1_b=[Q, C], ln3_g=[Q, C], ln3_b=[Q, C],
        xscale=[Q, NCOL], yscale=[Q, NCOL], wtab=[Q, NCOL],
        wm1=[Q, NCOL], wm2=[Q, NCOL], hm1=[Q, NCOL], hm2=[Q, NCOL],
        basetab=[Q, NCOL],
    )
    io = {}
    for name, shape in shapes.items():
        io[name] = nc.dram_tensor(name, shape, f32, kind="ExternalInput").ap()
    io["mask128"] = nc.dram_tensor("mask128", [128, Q], f32,
                                   kind="ExternalInput").ap()
    io["srcflat_bf"] = nc.dram_tensor("srcflat_bf", [S, C], mybir.dt.bfloat16,
                                      kind="ExternalInput").ap()
    io["replpat"] = nc.dram_tensor("replpat", [16, 128], f32,
                                   kind="ExternalInput").ap()
    for name, shape in [("val_wT_g", [C, HG * DH]), ("out_wT", [C, C]),
                        ("ffn_w1T", [C, DFF]), ("ffn_w2T", [DFF, C]),
                        ("sa_in_wT", [C, 3 * C]), ("sa_out_wT", [C, C]),
                        ("ea_in_wT", [C, 3 * C]), ("ea_out_wT", [C, C]),
                        ("offaw_wT", [C, 3 * NCOL])]:
        io[name] = nc.dram_tensor(name, shape, mybir.dt.bfloat16,
                                  kind="ExternalInput").ap()
    io["out"] = nc.dram_tensor("out", [Q, C], f32, kind="ExternalOutput").ap()

    with tile.TileContext(nc) as tc:
        for _ in range(loop):
            _emit(tc, io, use_ag=use_ag)
    nc.compile()
    return nc


def make_in_maps(inputs):
    """Build the 8 per-core input maps from the full problem inputs (numpy)."""
    import ml_dtypes
    inp = {k: np.ascontiguousarray(np.asarray(v, dtype=np.float32))
           if not k.startswith("src_") or k == "src" else np.asarray(v)
           for k, v in inputs.items()}
    lsi = np.asarray(inputs["src_level_start_index"]).astype(np.int64)
    spat = np.asarray(inputs["src_spatial_shapes"]).astype(np.int64)
    Wl = spat[:, 1].astype(np.float32)
    Hl = spat[:, 0].astype(np.float32)
    lcol = np.tile(np.repeat(np.arange(L), PTS), HG)  # [NCOL]
    rep = lambda row: np.ascontiguousarray(
        np.broadcast_to(row[None, :], (Q, NCOL)).astype(np.float32))
    mask = np.zeros((128, Q), np.float32)
    mask[np.arange(128), np.arange(128) % Q] = 1.0

    def wT(a):
        return np.ascontiguousarray(a.T.astype(np.float32))

    def row(a):
        return np.ascontiguousarray(a.reshape(1, -1).astype(np.float32))

    def repl(a):
        return np.ascontiguousarray(
            np.broadcast_to(a.reshape(1, -1), (Q, C)).astype(np.float32))

    in_maps = []
    for c in range(N_CORES):
        b, g = c // 2, c % 2
        heads = range(HG * g, HG * g + HG)
        vr = np.asarray(inp["src_valid_ratios"])[b]  # [L, 2]
        off_rows = np.concatenate([np.arange(h * L * PTS * 2, (h + 1) * L * PTS * 2)
                                   for h in heads])
        aw_rows = np.concatenate([np.arange(h * L * PTS, (h + 1) * L * PTS)
                                  for h in heads])
        offaw_w = np.concatenate([inp["ms_off_w"][off_rows],
                                  inp["ms_attn_w"][aw_rows]], axis=0)  # [180, C]
        offaw_b = np.concatenate([inp["ms_off_b"][off_rows],
                                  inp["ms_attn_b"][aw_rows]])
        vcols = np.concatenate([np.arange(h * DH, (h + 1) * DH) for h in heads])
        m = dict(
            tgt_in=np.ascontiguousarray(inp["tgt"][b]),
            extra_in=np.ascontiguousarray(inp["extra_memory"][b]),
            refpts=np.ascontiguousarray(inp["reference_points"][b]),
            srcflat_bf=np.ascontiguousarray(
                inp["src"][b].astype(ml_dtypes.bfloat16)),
            sa_in_wT=wT(inp["sa_in_w"]).astype(ml_dtypes.bfloat16),
            sa_in_b=row(inp["sa_in_b"]),
            sa_out_wT=wT(inp["sa_out_w"]).astype(ml_dtypes.bfloat16),
            sa_out_b=row(inp["sa_out_b"]),
            ea_in_wT=wT(inp["ea_in_w"]).astype(ml_dtypes.bfloat16),
            ea_in_b=row(inp["ea_in_b"]),
            ea_out_wT=wT(inp["ea_out_w"]).astype(ml_dtypes.bfloat16),
            ea_out_b=row(inp["ea_out_b"]),
            offaw_wT=wT(offaw_w).astype(ml_dtypes.bfloat16),
            offaw_b=row(offaw_b),
            val_wT_g=wT(inp["ms_val_w"][vcols]).astype(ml_dtypes.bfloat16),
            val_b_g=row(inp["ms_val_b"][vcols]),
            out_wT=wT(inp["ms_out_w"]).astype(ml_dtypes.bfloat16),
            out_b=row(inp["ms_out_b"]),
            ffn_w1T=wT(inp["ffn_w1"]).astype(ml_dtypes.bfloat16),
            ffn_b1=row(inp["ffn_b1"]),
            ffn_w2T=wT(inp["ffn_w2"]).astype(ml_dtypes.bfloat16),
            ffn_b2=row(inp["ffn_b2"]),
            ln2_g=repl(inp["ln2_g"]), ln2_b=repl(inp["ln2_b"]),
            lne_g=repl(inp["lne_g"]), lne_b=repl(inp["lne_b"]),
            ln1_g=repl(inp["ln1_g"]), ln1_b=repl(inp["ln1_b"]),
            ln3_g=repl(inp["ln3_g"]), ln3_b=repl(inp["ln3_b"]),
            xscale=rep(vr[lcol, 0] * Wl[lcol]),
            yscale=rep(vr[lcol, 1] * Hl[lcol]),
            wtab=rep(Wl[lcol]),
            wm1=rep(Wl[lcol] - 1), wm2=rep(Wl[lcol] - 2),
            hm1=rep(Hl[lcol] - 1), hm2=rep(Hl[lcol] - 2),
            basetab=rep(lsi[lcol].astype(np.float32)),
            mask128=mask,
            replpat=np.ascontiguousarray(
                (np.arange(128)[None, :] % 16 == np.arange(16)[:, None]
                 ).astype(np.float32)),
        )
        in_maps.append(m)
    return in_maps


def kernel(**inputs):
    from concourse.bass_utils import run_bass_kernel_spmd
    if "nc" not in _CACHE:
        _CACHE["nc"] = _build()
    nc = _CACHE["nc"]
    in_maps = make_in_maps(inputs)
    res = run_bass_kernel_spmd(nc, in_maps, core_ids=list(range(N_CORES)))
    out = np.zeros((B, Q, C), np.float32)
    for b in range(B):
        out[b] = res.results[2 * b]["out"]
    return out

